# revision 1
# baseline (speedup 1.0000x reference)
"""Trainium2 Bass kernel for nn_MessagePassing (gnn_message_passing) — final.

Math (per batch b = core b):
    coef[s,e] = sum_o adj[s,o] * edge[s,o,e]
    v[s,e,i]  = sum_j W[e,i,j] * node[s,j]
    out[s,i]  = sum_e coef[s,e] * v[s,e,i]

Design (batch-parallel over 8 cores; measured-rate driven):
  * All inputs stream on one upfront-issued SWDGE DMA ring (edge cast
    f32->bf16 inline; adj bf16 per-tile; node/W on the idle sync ring).
    Tiles 0 and 7 are split in halves at the ring's front/back to hide
    pipeline fill and tail; all descriptor-gen happens before DVE starts
    (the Q7 generator is locked out of the shared SBUF port by DVE ops).
  * ScalarE de-interleaves e-quads at int32 granularity (~4us/tile).
  * DVE runs the 8 per-e STT multiply-accumulates at 1 cycle/element.
  * PE computes v and sums the coef-scaled slices via identity-matmul
    PSUM accumulation; scale-ops split Sc/DVE by throttle-phase slack.
  * Endgame: the final 2MB chunk's de-interleave runs on DVE itself
    (immune to scheduler-hoisted ScalarE work) and a short DVE chain
    produces the last tile ~16us after the wire drains.
"""

import numpy as np
from contextlib import ExitStack

import concourse.bass as bass
import concourse.bacc as bacc
import concourse.mybir as mybir
import concourse.tile as tile
from concourse.bass_utils import run_bass_kernel_spmd
from concourse.masks import make_identity

B, N, D, E = 8, 1024, 128, 8
P = 128
NT = N // P  # 8 s-tiles per core
H = N // 2

F32 = mybir.dt.float32
BF16 = mybir.dt.bfloat16
I32 = mybir.dt.int32
MUL = mybir.AluOpType.mult
ADD = mybir.AluOpType.add
LAST = NT - 1


def build_nc():
    nc = bacc.Bacc("TRN2", target_bir_lowering=False, debug=False, num_devices=B)

    node_d = nc.dram_tensor("node_state", [N, D], F32, kind="ExternalInput").ap()
    edge_d = nc.dram_tensor("edge_type_mat", [N, N, E], F32, kind="ExternalInput").ap()
    adj_d = nc.dram_tensor("adj_mat", [N, N], F32, kind="ExternalInput").ap()
    w_d = nc.dram_tensor("W", [E, D, D], F32, kind="ExternalInput").ap()
    out_d = nc.dram_tensor("out", [N, D], F32, kind="ExternalOutput").ap()

    with tile.TileContext(nc) as tc, ExitStack() as ctx:
        const_pool = ctx.enter_context(tc.tile_pool(name="const", bufs=1))
        edge_pool = ctx.enter_context(tc.tile_pool(name="edge", bufs=8))
        quad_pool = ctx.enter_context(tc.tile_pool(name="quad", bufs=2))
        work_pool = ctx.enter_context(tc.tile_pool(name="work", bufs=2))
        coef_pool = ctx.enter_context(tc.tile_pool(name="coefp", bufs=3))
        psv_pool = ctx.enter_context(tc.tile_pool(name="psv", bufs=6, space="PSUM"))
        pss_pool = ctx.enter_context(tc.tile_pool(name="pss", bufs=2, space="PSUM"))

        # ---- all loads upfront on the SWDGE ring; ring order:
        #   e0a e0b a0 | e7a a7 | e1 a1 node w | e2 a2 ... e6 a6 | e7b ----
        edge_tiles = {}
        adj_r = adj_d.rearrange("(t p) o -> p t o", p=P)
        adj_tiles = [
            const_pool.tile([P, N], BF16, name=f"adj{t}") for t in range(NT)
        ]
        node_all = const_pool.tile([P, NT, D], F32)
        w_all = const_pool.tile([P, E, D], F32)  # [i, e, j]

        def load_edge(t, half=False):
            # each edge tile is followed on the ring by its adj slice
            et = edge_pool.tile([P, N, E], BF16, tag="edge_t")
            if not half:
                nc.gpsimd.dma_start(et[:], edge_d[bass.ts(t, P)])
            else:
                nc.gpsimd.dma_start(et[:, 0:H, :], edge_d[t * P : t * P + P, 0:H])
                nc.gpsimd.dma_start(et[:, H:N, :], edge_d[t * P : t * P + P, H:N])
            nc.gpsimd.dma_start(adj_tiles[t][:], adj_r[:, t, :])
            return et

        edge_tiles[0] = load_edge(0, half=True)
        # tile 7 first half + its adj ride at the FRONT of the ring
        et7 = edge_pool.tile([P, N, E], BF16, tag="edge_t", name="et7")
        nc.gpsimd.dma_start(et7[:, 0:H, :], edge_d[LAST * P : LAST * P + P, 0:H])
        nc.gpsimd.dma_start(adj_tiles[LAST][:], adj_r[:, LAST, :])
        edge_tiles[LAST] = et7
        S6 = NT - 2
        et6 = edge_pool.tile([P, N, E], BF16, tag="edge_t", name="et6")
        nc.gpsimd.dma_start(et6[:, 0:H, :], edge_d[S6 * P : S6 * P + P, 0:H])
        nc.gpsimd.dma_start(adj_tiles[S6][:], adj_r[:, S6, :])

        # ident built after the first issues (wire starts ~1.5us earlier)
        # but still early on the GpSimd queue, long before the remaining
        # dma_start instructions pace out with ring-space waits
        ident = const_pool.tile([P, P], F32)
        make_identity(nc, ident[:])
        ident_bf = const_pool.tile([P, P], BF16)
        nc.vector.tensor_copy(ident_bf[:], ident[:])

        # node/W on the idle sync ring: 1MB off the critical SWDGE wire;
        # they are not needed until ~25us so HWDGE starvation is harmless
        nc.sync.dma_start(node_all[:], node_d.rearrange("(t p) j -> p t j", p=P))
        nc.sync.dma_start(w_all[:], w_d.rearrange("e i j -> i e j"))

        edge_tiles[1] = load_edge(1)
        for t in range(2, NT - 2):
            edge_tiles[t] = load_edge(t)
        # tile 6 second half, then tile 7 second half as the last transfers
        nc.gpsimd.dma_start(et6[:, H:N, :], edge_d[S6 * P : S6 * P + P, H:N])
        nc.gpsimd.dma_start(et7[:, H:N, :], edge_d[LAST * P : LAST * P + P, H:N])

        # node^T[j, s] and W[e]^T[j, i] via PE transpose. The PSUM->SBUF
        # copies are deferred to DVE's early idle window (ScalarE is the
        # throttle-critical engine; 16 ACT copies there seed a cascade).
        nodeT = const_pool.tile([P, N], F32)
        wT = const_pool.tile([P, E, D], F32)  # [j, e, i]
        tcopies = []
        srcs = [(node_all[:, 0, :], nodeT[:, 0:P])]
        srcs += [(w_all[:, e, :], wT[:, e, :]) for e in range(E)]
        srcs += [(node_all[:, t, :], nodeT[:, bass.ts(t, P)]) for t in range(1, NT)]
        for src_ap, dst_ap in srcs:
            pt = pss_pool.tile([P, P], F32, tag="ps_small")
            nc.tensor.transpose(pt[:], src_ap, ident[:])
            tcopies.append((dst_ap, pt))

        coef7a = const_pool.tile([P, E], F32)
        coef7b = const_pool.tile([P, E], F32)
        coef6a = const_pool.tile([P, E], F32)
        coef6b = const_pool.tile([P, E], F32)

        state = {}

        def deint_and_stt(edge_t, t, k, halves, quad, coef_dst, deint_dve=False):
            """Quad de-interleave (Sc, or DVE for the final chunk) + 8 STTs."""
            h = N // halves
            deint_src = (
                edge_t[:, k * h : (k + 1) * h, :]
                .bitcast(I32)
                .rearrange("p n (q t) -> p q n t", q=2)
            )
            if deint_dve:
                # final chunk: DVE deints it so the op cannot queue behind
                # scheduler-hoisted scale-ops on ScalarE's endgame stream
                nc.vector.tensor_copy(
                    quad[:, :, k * h : (k + 1) * h, :].bitcast(I32), deint_src
                )
            else:
                nc.scalar.copy(
                    quad[:, :, k * h : (k + 1) * h, :].bitcast(I32), deint_src
                )
            scratch = work_pool.tile([P, N], BF16, tag="scratch")
            for e in range(E):
                q, j = divmod(e, 4)
                nc.vector.scalar_tensor_tensor(
                    out=scratch[:, 0:h],
                    in0=quad[:, q, k * h : (k + 1) * h, j],
                    scalar=1.0,
                    in1=adj_tiles[t][:, k * h : (k + 1) * h],
                    op0=MUL,
                    op1=MUL,
                    accum_out=coef_dst[:, e : e + 1],
                )

        def v_matmuls(t):
            psums = []
            for g in range(E // 4):
                pv = psv_pool.tile([P, 4, D], F32, tag="psum_v")
                nc.tensor.matmul(
                    pv[:],
                    lhsT=nodeT[:, bass.ts(t, P)],
                    rhs=wT[:, g * 4 : (g + 1) * 4, :],
                    start=True,
                    stop=True,
                )
                psums.append(pv)
            return psums

        def stage_compute(t):
            edge_t = edge_tiles.pop(t)
            halves = 2 if t == 0 else 1
            coef = coef_pool.tile([P, E], F32, tag="coef")
            coef_b = coef_pool.tile([P, E], F32, tag="coef_b")
            quad = quad_pool.tile([P, 2, N, 4], BF16, tag="quad")
            for k in range(halves):
                deint_and_stt(edge_t, t, k, halves, quad, coef if k == 0 else coef_b)
            if halves == 2:
                nc.vector.tensor_add(coef[:], coef[:], coef_b[:])
            state[t] = (coef, v_matmuls(t))

        def stage_reduce(t):
            """sv_e = v_e * coef_e (Sc + some DVE); PE psum-accumulates.

            For mid-stream tiles (t<=4) three scale-ops and the out-copy run
            on DVE, which has wire-wait slack there; ScalarE is the engine
            that stretches under compute-throttle and must stay underloaded.
            Endgame tiles keep everything on Sc so the DVE tail stays short.
            """
            coef, psums = state.pop(t)
            dve_share = 3 if t <= 4 else 0
            sv = work_pool.tile([P, E, D], BF16, tag="sv")
            for e in range(E):
                if e < dve_share:
                    nc.vector.tensor_scalar_mul(
                        sv[:, e, :], psums[e // 4][:, e % 4, :], coef[:, e : e + 1]
                    )
                else:
                    nc.scalar.mul(
                        sv[:, e, :], psums[e // 4][:, e % 4, :], coef[:, e : e + 1]
                    )
            acc = pss_pool.tile([P, D], F32, tag="ps_small")
            for e in range(E):
                nc.tensor.matmul(
                    acc[:],
                    lhsT=ident_bf[:],
                    rhs=sv[:, e, :],
                    start=(e == 0),
                    stop=(e == E - 1),
                )
            out_sb = work_pool.tile([P, D], F32, tag="out_sb")
            if t <= 4:
                nc.vector.tensor_copy(out_sb[:], acc[:])
            else:
                nc.scalar.copy(out_sb[:], acc[:])
            nc.sync.dma_start(out_d[bass.ts(t, P)], out_sb[:])

        # ---- software pipeline ----
        # tile 0 coef (deint + STTs) without v-matmuls: the transposed
        # nodeT/wT copies land on DVE between the early STT batches
        et0 = edge_tiles.pop(0)
        coef0 = work_pool.tile([P, E], F32, tag="coef")
        coef0_b = work_pool.tile([P, E], F32, tag="coef_b")
        quad0 = quad_pool.tile([P, 2, N, 4], BF16, tag="quad")
        deint_and_stt(et0, 0, 0, 2, quad0, coef0)
        deint_and_stt(et0, 0, 1, 2, quad0, coef0_b)
        nc.vector.tensor_add(coef0[:], coef0[:], coef0_b[:])
        # tile 7 first half: runs right after tile 0 while the wire streams
        quad7 = quad_pool.tile([P, 2, N, 4], BF16, tag="quad")
        deint_and_stt(et7, LAST, 0, 2, quad7, coef7a)
        for dst_ap, pt in tcopies:
            nc.vector.tensor_copy(dst_ap, pt[:])
        quad6 = quad_pool.tile([P, 2, N, 4], BF16, tag="quad")
        deint_and_stt(et6, S6, 0, 2, quad6, coef6a)
        state[0] = (coef0, v_matmuls(0))

        for t in range(1, NT - 2):
            stage_compute(t)
            if t >= 2:
                stage_reduce(t - 2)

        # tile 7 second half BEFORE the remaining reduces: its deint must
        # not queue behind scales on ScalarE. v_matmuls(7) stays after
        # reduce(4) (PE is in-order: v7 needs the PSUM bank accum4 frees).
        quad6b = quad_pool.tile([P, 2, N, 4], BF16, tag="quad")
        deint_and_stt(et6, S6, 1, 2, quad6b, coef6b, deint_dve=True)
        nc.vector.tensor_add(coef6b[:], coef6b[:], coef6a[:])
        stage_reduce(4)
        state[S6] = (coef6b, v_matmuls(S6))
        quad7b = quad_pool.tile([P, 2, N, 4], BF16, tag="quad")
        deint_and_stt(et7, LAST, 1, 2, quad7b, coef7b, deint_dve=True)
        nc.vector.tensor_add(coef7b[:], coef7b[:], coef7a[:])
        psums7 = v_matmuls(LAST)
        stage_reduce(NT - 3)
        stage_reduce(NT - 2)

        # shortest-tail reduce for tile 7: direct DVE chain
        acc_a = work_pool.tile([P, D], F32, tag="acc_a")
        acc_b = work_pool.tile([P, D], F32, tag="acc_b")
        nc.vector.tensor_scalar_mul(acc_a[:], psums7[0][:, 0, :], coef7b[:, 0:1])
        cur, nxt = acc_a, acc_b
        for e in range(1, E):
            nc.vector.scalar_tensor_tensor(
                out=nxt[:],
                in0=psums7[e // 4][:, e % 4, :],
                scalar=coef7b[:, e : e + 1],
                in1=cur[:],
                op0=MUL,
                op1=ADD,
            )
            cur, nxt = nxt, cur
        nc.sync.dma_start(out_d[bass.ts(LAST, P)], cur[:])

    nc.compile()
    return nc


_NC_CACHE = None


def get_nc():
    global _NC_CACHE
    if _NC_CACHE is None:
        _NC_CACHE = build_nc()
    return _NC_CACHE


def make_in_maps(node_state, edge_type_mat, adj_mat, W):
    return [
        {
            "node_state": np.ascontiguousarray(node_state[b], dtype=np.float32),
            "edge_type_mat": np.ascontiguousarray(edge_type_mat[b], dtype=np.float32),
            "adj_mat": np.ascontiguousarray(adj_mat[b], dtype=np.float32),
            "W": np.ascontiguousarray(W, dtype=np.float32),
        }
        for b in range(B)
    ]


def kernel(node_state, edge_type_mat, adj_mat, W):
    nc = get_nc()
    in_maps = make_in_maps(node_state, edge_type_mat, adj_mat, W)
    res = run_bass_kernel_spmd(nc, in_maps, list(range(B)))
    return np.stack([res.results[b]["out"] for b in range(B)], axis=0)

